# revision 1
# baseline (speedup 1.0000x reference)
"""GCN (3x ChebConv K=3 + global mean pool + linear head) on 8 Trainium2
NeuronCores via Bass/Tile.

Sharding: nodes (and their incident in-edges) are partitioned into 8
contiguous dst ranges. ChebConv is rewritten (K=3):
    out = X(W0 - W2) + T1 W1 + L(T1 (2 W2)),  T1 = L X
with L = -D^-1/2 A D^-1/2 applied as: row-scale by dinv on the input table,
ea-weighted gather(by src)/scatter-add(by dst), row-scale by -dinv on the
output. The gathered tables (X, P=T1(2W2), h) are replicated per-core via
AllGather after each local production step; pooled sums use one small
AllReduce. Sparse propagate uses the GPSIMD ucode dma_gather/dma_scatter_add
(int16 indices; tables split at row 32768 into lo/hi views). Edges are dealt
round-robin from dst-sorted order so a 1024-edge scatter group never repeats
a dst row.
"""
import sys
sys.path.insert(0, "/opt/trn_rl_repo")
import numpy as np

P = 128
NCORES = 8
R = 1024  # edges per gather/scatter group


def _wrap16_groups(groups, cap_groups):
    """groups: list of 1-d int arrays (each <= R). Returns [128, cap_groups*R/16]
    int16 in the ucode 16-partition wrap, replicated to 128 partitions."""
    buf = np.zeros((cap_groups, R), np.int64)
    for i, a in enumerate(groups):
        buf[i, :len(a)] = a
    flat = buf.reshape(-1)
    w = np.ascontiguousarray(flat.reshape(-1, 16).T).astype(np.int16)
    return np.tile(w, (8, 1))


def _wrap128_groups(groups, cap_groups):
    buf = np.zeros((cap_groups, R), np.float32)
    for i, a in enumerate(groups):
        buf[i, :len(a)] = a
    flat = buf.reshape(-1)
    return np.ascontiguousarray(flat.reshape(-1, P).T)


def _prep(inputs):
    x = np.asarray(inputs["x"], np.float32)
    ei = np.asarray(inputs["edge_index"]).astype(np.int64)
    ea = np.asarray(inputs["edge_attr"], np.float32)
    batch = np.asarray(inputs["batch"]).astype(np.int64)
    Ws = [np.asarray(inputs[k], np.float32) for k in ("W1", "W2", "W3")]
    bs = [np.asarray(inputs[k], np.float32) for k in ("b1", "b2", "b3")]
    Wl = np.asarray(inputs["Wl"], np.float32)
    bl = np.asarray(inputs["bl"], np.float32)

    N, FIN = x.shape
    RPC = N // NCORES
    NB = (RPC + P - 1) // P
    RB = NB * P
    NTOT = RB * NCORES
    XCOLS = ((FIN + 1 + 63) // 64) * 64
    F1, F2, F3 = Ws[0].shape[2], Ws[1].shape[2], Ws[2].shape[2]
    PW3 = ((F3 + 63) // 64) * 64

    src, dst = ei[0], ei[1]
    owner = dst // RPC
    g = (src // RPC) * RB + (src % RPC)
    dstloc = dst % RPC
    src_owner = src // RPC
    srcloc = src % RPC

    # per-core edge sets, dst-sorted
    lo_sets, hi_sets, deg_sets = [], [], []
    for c in range(NCORES):
        m = np.nonzero(owner == c)[0]
        m = m[np.argsort(dstloc[m], kind="stable")]
        gs = g[m]
        lo_sets.append(m[gs < 32768])
        hi_sets.append(m[gs >= 32768])
        md = np.nonzero(src_owner == c)[0]
        md = md[np.argsort(srcloc[md], kind="stable")]
        deg_sets.append(md)

    def ngroups(sets):
        return max((len(s) + R - 1) // R for s in sets)
    NLO_G = max(1, ngroups(lo_sets))
    NHI_G = ngroups(hi_sets)
    NDEG_G = max(1, ngroups(deg_sets))

    xtbl = np.zeros((NTOT, XCOLS), np.float32)
    for c in range(NCORES):
        xtbl[c * RB:c * RB + RPC, :FIN] = x[c * RPC:(c + 1) * RPC]
    iota = np.tile(np.arange(P, dtype=np.float32), (P, 1))
    ident = np.eye(P, dtype=np.float32)

    in_maps = []
    for c in range(NCORES):
        def deal(m, ng):
            return [m[i::ng] for i in range(ng)] if ng > 0 and len(m) else \
                   [np.zeros(0, np.int64) for _ in range(ng)]
        lo_g = deal(lo_sets[c], NLO_G)
        hi_g = deal(hi_sets[c], NHI_G)
        dg_g = deal(deg_sets[c], NDEG_G)
        all_g = lo_g + hi_g

        im = dict(
            xtbl=xtbl,
            xT=np.zeros((FIN, RB), np.float32),
            glo=_wrap16_groups([g[m] for m in lo_g], NLO_G),
            ghi=_wrap16_groups([g[m] - 32768 for m in hi_g], max(NHI_G, 1)),
            dsti=_wrap16_groups([dstloc[m] for m in all_g], NLO_G + max(NHI_G, 1)),
            eav=_wrap128_groups([ea[m] for m in all_g], NLO_G + max(NHI_G, 1)),
            dsrc=_wrap16_groups([srcloc[m] for m in dg_g], NDEG_G),
            dea=_wrap128_groups([ea[m] for m in dg_g], NDEG_G),
            w1=Ws[0], w2=Ws[1], w3=Ws[2], wl=Wl,
            b1b=np.tile(bs[0], (P, 1)), b2b=np.tile(bs[1], (P, 1)),
            b3b=np.tile(bs[2], (P, 1)), blb=np.tile(bl, (P, 1)),
            batchf=np.full((P, NB), 999.0, np.float32),
            iota=iota, ident=ident,
        )
        im["xT"][:, :RPC] = x[c * RPC:(c + 1) * RPC].T
        blp = np.full(RB, 999.0, np.float32)
        blp[:RPC] = batch[c * RPC:(c + 1) * RPC]
        im["batchf"] = np.ascontiguousarray(blp.reshape(NB, P).T)
        in_maps.append(im)

    hp = dict(N=N, FIN=FIN, F1=F1, F2=F2, F3=F3, RPC=RPC, NB=NB, RB=RB,
              NTOT=NTOT, XCOLS=XCOLS, PW3=PW3, NLO_G=NLO_G, NHI_G=NHI_G,
              NDEG_G=NDEG_G)
    return hp, in_maps


def _build(hp, debug=False):
    import concourse.bass as bass
    import concourse.bacc as bacc
    import concourse.tile as tile
    import concourse.mybir as mybir
    from concourse import library_config
    dt = mybir.dt
    AF = mybir.ActivationFunctionType
    OP = mybir.AluOpType

    FIN, F1, F2, F3 = hp["FIN"], hp["F1"], hp["F2"], hp["F3"]
    NB, RB, NTOT = hp["NB"], hp["RB"], hp["NTOT"]
    XCOLS, PW3 = hp["XCOLS"], hp["PW3"]
    NLO_G, NHI_G, NDEG_G = hp["NLO_G"], hp["NHI_G"], hp["NDEG_G"]
    NG_ALL = NLO_G + max(NHI_G, 1)
    NCOLT = NTOT // P
    f32 = dt.float32

    nc = bacc.Bacc("TRN2", target_bir_lowering=False, debug=False,
                   num_devices=NCORES, dynamic_dma_scratch_size=24576)

    xtbl = nc.dram_tensor("xtbl", [NTOT, XCOLS], f32, kind="ExternalInput")
    xT = nc.dram_tensor("xT", [FIN, RB], f32, kind="ExternalInput")
    glo = nc.dram_tensor("glo", [P, NLO_G * R // 16], dt.int16,
                         kind="ExternalInput")
    ghi = nc.dram_tensor("ghi", [P, max(NHI_G, 1) * R // 16], dt.int16,
                         kind="ExternalInput")
    dsti = nc.dram_tensor("dsti", [P, NG_ALL * R // 16], dt.int16,
                          kind="ExternalInput")
    eav = nc.dram_tensor("eav", [P, NG_ALL * R // P], f32,
                         kind="ExternalInput")
    dsrc = nc.dram_tensor("dsrc", [P, NDEG_G * R // 16], dt.int16,
                          kind="ExternalInput")
    dea = nc.dram_tensor("dea", [P, NDEG_G * R // P], f32,
                         kind="ExternalInput")
    w1 = nc.dram_tensor("w1", [3, FIN, F1], f32, kind="ExternalInput")
    w2 = nc.dram_tensor("w2", [3, F1, F2], f32, kind="ExternalInput")
    w3 = nc.dram_tensor("w3", [3, F2, F3], f32, kind="ExternalInput")
    wl = nc.dram_tensor("wl", [F3, 2], f32, kind="ExternalInput")
    b1b = nc.dram_tensor("b1b", [P, F1], f32, kind="ExternalInput")
    b2b = nc.dram_tensor("b2b", [P, F2], f32, kind="ExternalInput")
    b3b = nc.dram_tensor("b3b", [P, F3], f32, kind="ExternalInput")
    blb = nc.dram_tensor("blb", [P, 2], f32, kind="ExternalInput")
    batchf = nc.dram_tensor("batchf", [P, NB], f32, kind="ExternalInput")
    iota = nc.dram_tensor("iota", [P, P], f32, kind="ExternalInput")
    ident = nc.dram_tensor("ident", [P, P], f32, kind="ExternalInput")
    y = nc.dram_tensor("y", [P, 2], f32, kind="ExternalOutput")
    if debug:
        dbgs = {k: nc.dram_tensor(k, [P, w], f32, kind="ExternalOutput")
                for k, w in [("dbg_deg", hp["NB"]), ("dbg_ya1", hp["XCOLS"]),
                             ("dbg_p1", hp["F1"]), ("dbg_yb1", hp["F1"]),
                             ("dbg_h1", hp["F1"]), ("dbg_pool", hp["F3"] + 1),
                             ("dbg_dinv", hp["NB"])]}

    with tile.TileContext(nc) as tc:
        with tc.tile_pool(name="const", bufs=1) as cst, \
             tc.tile_pool(name="slab", bufs=2) as slb, \
             tc.tile_pool(name="slab1", bufs=1) as slb1, \
             tc.tile_pool(name="once", bufs=1) as wk1, \
             tc.tile_pool(name="work", bufs=3) as wk, \
             tc.tile_pool(name="vbuf", bufs=3) as vb, \
             tc.tile_pool(name="idxp", bufs=3) as ixp, \
             tc.tile_pool(name="psmm", bufs=3, space="PSUM") as psmm, \
             tc.tile_pool(name="pstr", bufs=2, space="PSUM") as pstr, \
             tc.tile_pool(name="psfin", bufs=1, space="PSUM") as psfin, \
             tc.tile_pool(name="dram", bufs=1, space="DRAM") as dram:

            nc.gpsimd.load_library(library_config.mlp)

            def dtile(name, shape):
                return dram.tile(shape, f32, tag=name, name=name)
            xint = dtile("xint", [NTOT, XCOLS])
            ya1 = dtile("ya1", [RB, XCOLS]); yb1 = dtile("yb1", [RB, F1])
            ya2 = dtile("ya2", [RB, F1]);    yb2 = dtile("yb2", [RB, F2])
            ya3 = dtile("ya3", [RB, F2]);    yb3 = dtile("yb3", [RB, PW3])
            p1s = dtile("p1s", [RB, F1]); p1f = dtile("p1f", [NTOT, F1])
            h1s = dtile("h1s", [RB, F1]); h1f = dtile("h1f", [NTOT, F1])
            p2s = dtile("p2s", [RB, F2]); p2f = dtile("p2f", [NTOT, F2])
            h2s = dtile("h2s", [RB, F2]); h2f = dtile("h2f", [NTOT, F2])
            p3s = dtile("p3s", [RB, PW3]); p3f = dtile("p3f", [NTOT, PW3])
            degt = dtile("degt", [RB, 64])
            degsh = dtile("degsh", [RB, 1]); degf = dtile("degf", [NTOT, 1])
            arin = dtile("arin", [P, F3 + 1]); arout = dtile("arout", [P, F3 + 1])

            # ---------- constants ----------
            identt = cst.tile([P, P], f32)
            nc.sync.dma_start(out=identt[:], in_=ident[:, :])
            iotat = cst.tile([P, P], f32)
            nc.sync.dma_start(out=iotat[:], in_=iota[:, :])
            b1t = cst.tile([P, F1], f32)
            nc.sync.dma_start(out=b1t[:], in_=b1b[:, :])
            b2t = cst.tile([P, F2], f32)
            nc.sync.dma_start(out=b2t[:], in_=b2b[:, :])
            b3t = cst.tile([P, F3], f32)
            nc.sync.dma_start(out=b3t[:], in_=b3b[:, :])
            blt = cst.tile([P, 2], f32)
            nc.sync.dma_start(out=blt[:], in_=blb[:, :])
            batcht = cst.tile([P, NB], f32)
            nc.sync.dma_start(out=batcht[:], in_=batchf[:, :])
            eat = cst.tile([P, NG_ALL * R // P], f32)
            nc.sync.dma_start(out=eat[:], in_=eav[:, :])
            deat = cst.tile([P, NDEG_G * R // P], f32)
            nc.sync.dma_start(out=deat[:], in_=dea[:, :])
            ztile = cst.tile([P, 512], f32)
            nc.vector.memset(ztile[:], 0.0)

            # weight slabs
            def wslabs(wt, name, fin, fout):
                ks = []
                for k in range(3):
                    parts = []
                    for o in range(0, fin, P):
                        kp = min(P, fin - o)
                        t = cst.tile([P, fout], f32, tag=f"w_{name}_{k}_{o}")
                        nc.sync.dma_start(out=t[:kp, :], in_=wt[k, o:o + kp, :])
                        parts.append((t, kp))
                    ks.append(parts)
                wa, wc2 = [], []
                for o_i, (t0, kp) in enumerate(ks[0]):
                    t2 = ks[2][o_i][0]
                    ta = cst.tile([P, fout], f32, tag=f"wa_{name}_{o_i}")
                    nc.vector.tensor_tensor(out=ta[:kp, :], in0=t0[:kp, :],
                                            in1=t2[:kp, :], op=OP.subtract)
                    wa.append((ta, kp))
                    tcv = cst.tile([P, fout], f32, tag=f"wc_{name}_{o_i}")
                    nc.vector.tensor_scalar_mul(tcv[:kp, :], t2[:kp, :], 2.0)
                    wc2.append((tcv, kp))
                return dict(wa=wa, w1=ks[1], wc2=wc2)
            W1s = wslabs(w1, "w1", FIN, F1)
            W2s = wslabs(w2, "w2", F1, F2)
            W3s = wslabs(w3, "w3", F2, F3)
            wlt = cst.tile([P, 2], f32)
            nc.sync.dma_start(out=wlt[:F3, :], in_=wl[:, :])

            # ---------- copy xtbl -> xint, zero Y tables ----------
            xfl_in = xtbl[:, :].rearrange("(p a) f -> p (a f)", p=P)
            xfl_out = xint[:].rearrange("(p a) f -> p (a f)", p=P)
            CW = 1024
            for c0 in range(0, xfl_in.shape[1], CW):
                w_ = min(CW, xfl_in.shape[1] - c0)
                t = wk.tile([P, CW], f32, tag="xcopy")
                nc.sync.dma_start(out=t[:, :w_], in_=xfl_in[:, c0:c0 + w_])
                nc.sync.dma_start(out=xfl_out[:, c0:c0 + w_], in_=t[:, :w_])
            for tbl in [ya1, yb1, ya2, yb2, ya3, yb3, p3s, degt]:
                flat = tbl[:].rearrange("(p a) f -> p (a f)", p=P)
                tot = flat.shape[1]
                for c0 in range(0, tot, 512):
                    w_ = min(512, tot - c0)
                    nc.sync.dma_start(out=flat[:, c0:c0 + w_],
                                      in_=ztile[:, :w_])

            # ---------- generic propagate ----------
            def spmm(src_full, ytbl, fcols, dinv_col):
                for grp in range(NLO_G + NHI_G):
                    is_hi = grp >= NLO_G
                    garr, gi = (ghi, grp - NLO_G) if is_hi else (glo, grp)
                    it = ixp.tile([P, R // 16], dt.int16, tag="sp_it")
                    nc.sync.dma_start(
                        out=it[:],
                        in_=garr[:, gi * (R // 16):(gi + 1) * (R // 16)])
                    vt = vb.tile([P, R // P, fcols], f32, tag="vt")
                    va = vt[:]
                    src_ap = src_full[32768:, :] if is_hi else src_full[:, :]
                    nc.gpsimd.dma_gather(
                        out_ap=va, in_ap=src_ap, idxs_ap=it[:],
                        num_idxs=R, num_idxs_reg=R, elem_size=fcols,
                        single_packet=False)
                    esl = eat[:, grp * (R // P):(grp + 1) * (R // P)]
                    if dinv_col is not None:
                        esc = wk.tile([P, R // P], f32, tag="esc")
                        nc.vector.tensor_tensor(
                            out=esc[:, :, None],
                            in0=vt[:, :, dinv_col:dinv_col + 1],
                            in1=esl[:, :, None], op=OP.mult)
                        esl = esc[:]
                    nc.vector.tensor_tensor(
                        out=va, in0=va,
                        in1=esl[:, :, None].to_broadcast([P, R // P, fcols]),
                        op=OP.mult)
                    dit = ixp.tile([P, R // 16], dt.int16, tag="sp_dit")
                    nc.sync.dma_start(
                        out=dit[:],
                        in_=dsti[:, grp * (R // 16):(grp + 1) * (R // 16)])
                    nc.gpsimd.dma_scatter_add(
                        out_ap=ytbl[:], in_ap=va, idxs_ap=dit[:],
                        num_idxs=R, num_idxs_reg=R, elem_size=fcols,
                        single_packet=False)

            # ---------- deg ----------
            vdeg = cst.tile([P, R // P, 64], f32)
            nc.vector.memset(vdeg[:], 0.0)
            for grp in range(NDEG_G):
                it = ixp.tile([P, R // 16], dt.int16, tag="dg_it")
                nc.sync.dma_start(
                    out=it[:],
                    in_=dsrc[:, grp * (R // 16):(grp + 1) * (R // 16)])
                nc.vector.tensor_copy(
                    out=vdeg[:, :, 0:1],
                    in_=deat[:, grp * (R // P):(grp + 1) * (R // P), None])
                nc.gpsimd.dma_scatter_add(
                    out_ap=degt[:], in_ap=vdeg[:], idxs_ap=it[:],
                    num_idxs=R, num_idxs_reg=R, elem_size=64,
                    single_packet=False)

            degsb = wk1.tile([P, NB], f32, tag="degsb")
            nc.sync.dma_start(
                out=degsb[:],
                in_=degt[:, 0:1].rearrange("(b p) c -> p (b c)", p=P))
            nc.sync.dma_start(
                out=degsh[:].rearrange("(b p) c -> p (b c)", p=P),
                in_=degsb[:])
            nc.gpsimd.collective_compute(
                "AllGather", OP.bypass, replica_groups=[list(range(NCORES))],
                ins=[degsh.opt()], outs=[degf.opt()])

            def dinv_of(deg_ap, cols, tag):
                m = wk1.tile([P, cols], f32, tag=tag + "m")
                nc.vector.tensor_scalar(out=m[:], in0=deg_ap, scalar1=0.0,
                                        scalar2=None, op0=OP.is_le)
                safe = wk1.tile([P, cols], f32, tag=tag + "s")
                nc.vector.tensor_tensor(out=safe[:], in0=deg_ap, in1=m[:],
                                        op=OP.add)
                sq = wk1.tile([P, cols], f32, tag=tag + "q")
                nc.scalar.activation(out=sq[:], in_=safe[:], func=AF.Sqrt)
                rcp = wk1.tile([P, cols], f32, tag=tag + "r")
                nc.vector.reciprocal(rcp[:], sq[:])
                gm = wk1.tile([P, cols], f32, tag=tag + "g")
                nc.vector.tensor_scalar(out=gm[:], in0=deg_ap, scalar1=0.0,
                                        scalar2=None, op0=OP.is_gt)
                dv = cst.tile([P, cols], f32, tag=tag + "d")
                nc.vector.tensor_tensor(out=dv[:], in0=rcp[:], in1=gm[:],
                                        op=OP.mult)
                return dv

            if debug:
                nc.sync.dma_start(out=dbgs["dbg_deg"][:, :], in_=degsb[:])
            dinv_own = dinv_of(degsb[:], NB, "down")
            if debug:
                nc.sync.dma_start(out=dbgs["dbg_dinv"][:, :], in_=dinv_own[:])
            negd = cst.tile([P, NB], f32)
            nc.vector.tensor_scalar_mul(negd[:], dinv_own[:], -1.0)
            neg2d = cst.tile([P, NB], f32)
            nc.vector.tensor_scalar_mul(neg2d[:], dinv_own[:], -2.0)

            degfsb = wk1.tile([P, NCOLT], f32, tag="degfsb")
            nc.sync.dma_start(
                out=degfsb[:],
                in_=degf[:, 0:1].rearrange("(b p) c -> p (b c)", p=P))
            dinv_full = dinv_of(degfsb[:], NCOLT, "dfull")
            nc.sync.dma_start(
                out=xint[:, FIN:FIN + 1].rearrange("(b p) c -> p (b c)", p=P),
                in_=dinv_full[:])

            # ---------- dense helpers ----------
            def transpose_to_slab(src_sbuf, fin, slabA, slabB, nb):
                k0 = min(P, fin)
                pt = pstr.tile([P, P], f32, tag="ptr")
                nc.tensor.transpose(out=pt[:k0, :], in_=src_sbuf[:, 0:k0],
                                    identity=identt[:])
                nc.scalar.activation(out=slabA[:k0, nb * P:(nb + 1) * P],
                                     in_=pt[:k0, :], func=AF.Copy)
                if fin > P:
                    pt2 = pstr.tile([P, P], f32, tag="ptr")
                    nc.tensor.transpose(out=pt2[:fin - P, :],
                                        in_=src_sbuf[:, P:fin],
                                        identity=identt[:])
                    nc.scalar.activation(
                        out=slabB[:fin - P, nb * P:(nb + 1) * P],
                        in_=pt2[:fin - P, :], func=AF.Copy)

            def lhs_slices(slabA, slabB, fin, nb):
                res = [(slabA[:min(P, fin), nb * P:(nb + 1) * P], min(P, fin))]
                if fin > P:
                    res.append((slabB[:fin - P, nb * P:(nb + 1) * P], fin - P))
                return res

            def mm_acc(ps, lhs_list, w_parts, start, stop):
                n = len(lhs_list)
                for i, ((lap, kp), (wt_, kpw)) in enumerate(
                        zip(lhs_list, w_parts)):
                    assert kp == kpw, (kp, kpw)
                    nc.tensor.matmul(ps, lap, wt_[:kp, :],
                                     start=(start and i == 0),
                                     stop=(stop and i == n - 1))

            layer_cfg = [
                dict(fin=FIN, fout=F1, xc=XCOLS, Ws=W1s, bt=b1t,
                     ya=ya1, yb=yb1, ps=p1s, pf=p1f, hs=h1s, hf=h1f,
                     pw=F1, dcol=FIN, xsrc=xint),
                dict(fin=F1, fout=F2, xc=F1, Ws=W2s, bt=b2t,
                     ya=ya2, yb=yb2, ps=p2s, pf=p2f, hs=h2s, hf=h2f,
                     pw=F2, dcol=None, xsrc=h1f),
                dict(fin=F2, fout=F3, xc=F2, Ws=W3s, bt=b3t,
                     ya=ya3, yb=yb3, ps=p3s, pf=p3f, hs=None, hf=None,
                     pw=PW3, dcol=None, xsrc=h2f),
            ]

            pooled = psfin.tile([P, F3 + 1], f32, tag="pooled")
            prev_slabA = prev_slabB = None

            for li, cfg in enumerate(layer_cfg):
                fin, fout = cfg["fin"], cfg["fout"]
                spmm(cfg["xsrc"], cfg["ya"], cfg["xc"], cfg["dcol"])

                t1A = slb1.tile([P, NB * P], f32, tag="t1A")
                t1B = (slb1.tile([P, NB * P], f32, tag="t1B", name="t1B")
                       if fin > P else None)
                for nb in range(NB):
                    yat = wk.tile([P, fin], f32, tag="yat")
                    nc.sync.dma_start(
                        out=yat[:], in_=cfg["ya"][nb * P:(nb + 1) * P, 0:fin])
                    t1 = wk.tile([P, fin], f32, tag="t1")
                    nc.scalar.activation(out=t1[:], in_=yat[:], func=AF.Copy,
                                         scale=negd[:, nb:nb + 1])
                    transpose_to_slab(t1, fin, t1A, t1B, nb)
                    psP = psmm.tile([P, fout], f32, tag="ps")
                    mm_acc(psP[:], lhs_slices(t1A, t1B, fin, nb),
                           cfg["Ws"]["wc2"], start=True, stop=True)
                    pst = wk.tile([P, fout], f32, tag="pst")
                    nc.scalar.activation(out=pst[:], in_=psP[:], func=AF.Copy,
                                         scale=dinv_own[:, nb:nb + 1])
                    nc.sync.dma_start(
                        out=cfg["ps"][nb * P:(nb + 1) * P, 0:fout], in_=pst[:])
                nc.gpsimd.collective_compute(
                    "AllGather", OP.bypass,
                    replica_groups=[list(range(NCORES))],
                    ins=[cfg["ps"].opt()], outs=[cfg["pf"].opt()])

                spmm(cfg["pf"], cfg["yb"], cfg["pw"], None)
                if debug and li == 0:
                    dt1 = wk.tile([P, XCOLS], f32, tag="dt1")
                    nc.sync.dma_start(out=dt1[:], in_=ya1[0:P, :])
                    nc.sync.dma_start(out=dbgs["dbg_ya1"][:, :], in_=dt1[:])
                    dt2 = wk.tile([P, F1], f32, tag="dt2")
                    nc.sync.dma_start(out=dt2[:], in_=p1f[0:P, 0:F1])
                    nc.sync.dma_start(out=dbgs["dbg_p1"][:, :], in_=dt2[:])
                    dt3 = wk.tile([P, F1], f32, tag="dt3")
                    nc.sync.dma_start(out=dt3[:], in_=yb1[0:P, 0:F1])
                    nc.sync.dma_start(out=dbgs["dbg_yb1"][:, :], in_=dt3[:])

                newA = (slb.tile([P, NB * P], f32, tag="hA", name="hA")
                        if li < 2 else None)
                for nb in range(NB):
                    psO = psmm.tile([P, fout], f32, tag="ps")
                    if li == 0:
                        xa = wk.tile([P, P], f32, tag="xa")
                        nc.sync.dma_start(out=xa[:],
                                          in_=xT[0:P, nb * P:(nb + 1) * P])
                        xlhs = [(xa[:, :], P)]
                        if fin > P:
                            xb = wk.tile([P, P], f32, tag="xb")
                            nc.sync.dma_start(
                                out=xb[:fin - P, :],
                                in_=xT[P:fin, nb * P:(nb + 1) * P])
                            xlhs.append((xb[:fin - P, :], fin - P))
                    else:
                        xlhs = lhs_slices(prev_slabA, prev_slabB, fin, nb)
                    mm_acc(psO[:], xlhs, cfg["Ws"]["wa"], start=True,
                           stop=False)
                    mm_acc(psO[:], lhs_slices(t1A, t1B, fin, nb),
                           cfg["Ws"]["w1"], start=False, stop=True)
                    ybt = wk.tile([P, fout], f32, tag="ybt")
                    nc.sync.dma_start(
                        out=ybt[:], in_=cfg["yb"][nb * P:(nb + 1) * P, 0:fout])
                    yscl = wk.tile([P, fout], f32, tag="yscl")
                    nc.scalar.activation(out=yscl[:], in_=ybt[:], func=AF.Copy,
                                         scale=negd[:, nb:nb + 1])
                    s1 = wk.tile([P, fout], f32, tag="s1")
                    nc.vector.tensor_tensor(out=s1[:], in0=psO[:],
                                            in1=yscl[:], op=OP.add)
                    s2 = wk.tile([P, fout], f32, tag="s2")
                    nc.vector.tensor_tensor(out=s2[:], in0=s1[:],
                                            in1=cfg["bt"][:], op=OP.add)
                    h = wk.tile([P, fout], f32, tag="h")
                    nc.scalar.activation(out=h[:], in_=s2[:], func=AF.Relu)
                    if li < 2:
                        hs = wk.tile([P, fout], f32, tag="hs")
                        nc.scalar.activation(out=hs[:], in_=h[:], func=AF.Copy,
                                             scale=dinv_own[:, nb:nb + 1])
                        nc.sync.dma_start(
                            out=cfg["hs"][nb * P:(nb + 1) * P, 0:fout],
                            in_=hs[:])
                        transpose_to_slab(h, fout, newA, None, nb)
                    else:
                        r33 = wk.tile([P, F3 + 1], f32, tag="r33")
                        nc.vector.memset(r33[:], 1.0)
                        nc.vector.tensor_copy(out=r33[:, 0:F3], in_=h[:])
                        B = wk.tile([P, P], f32, tag="Bsel")
                        nc.vector.tensor_scalar(
                            out=B[:], in0=iotat[:],
                            scalar1=batcht[:, nb:nb + 1], scalar2=None,
                            op0=OP.is_equal)
                        nc.tensor.matmul(pooled[:], B[:], r33[:],
                                         start=(nb == 0), stop=(nb == NB - 1))
                if li < 2:
                    nc.gpsimd.collective_compute(
                        "AllGather", OP.bypass,
                        replica_groups=[list(range(NCORES))],
                        ins=[cfg["hs"].opt()], outs=[cfg["hf"].opt()])
                    prev_slabA, prev_slabB = newA, None
                    if debug and li == 0:
                        dt4 = wk.tile([P, F1], f32, tag="dt4")
                        nc.sync.dma_start(out=dt4[:], in_=h1f[0:P, 0:F1])
                        nc.sync.dma_start(out=dbgs["dbg_h1"][:, :], in_=dt4[:])

            # ---------- pooled mean + head ----------
            psb = wk1.tile([P, F3 + 1], f32, tag="psb")
            nc.vector.tensor_copy(out=psb[:], in_=pooled[:])
            if debug:
                nc.sync.dma_start(out=dbgs["dbg_pool"][:, :], in_=psb[:])
            nc.sync.dma_start(out=arin[:, :], in_=psb[:])
            nc.gpsimd.collective_compute(
                "AllReduce", OP.add, replica_groups=[list(range(NCORES))],
                ins=[arin.opt()], outs=[arout.opt()])
            pr = wk1.tile([P, F3 + 1], f32, tag="pr")
            nc.sync.dma_start(out=pr[:], in_=arout[:, :])
            cmax = wk1.tile([P, 1], f32, tag="cmax")
            nc.vector.tensor_scalar_max(cmax[:], pr[:, F3:F3 + 1], 1.0)
            rcp = wk1.tile([P, 1], f32, tag="rcpf")
            nc.vector.reciprocal(rcp[:], cmax[:])
            pm = wk1.tile([P, F3], f32, tag="pm")
            nc.scalar.activation(out=pm[:], in_=pr[:, 0:F3], func=AF.Copy,
                                 scale=rcp[:, 0:1])
            ptp = pstr.tile([P, P], f32, tag="ptr")
            nc.tensor.transpose(out=ptp[:F3, :], in_=pm[:], identity=identt[:])
            pmT = wk1.tile([P, P], f32, tag="pmT")
            nc.scalar.activation(out=pmT[:F3, :], in_=ptp[:F3, :], func=AF.Copy)
            psy = psfin.tile([P, 2], f32, tag="psy")
            nc.tensor.matmul(psy[:], pmT[:F3, :], wlt[:F3, :], start=True,
                             stop=True)
            yt = wk1.tile([P, 2], f32, tag="yt")
            nc.vector.tensor_tensor(out=yt[:], in0=psy[:], in1=blt[:],
                                    op=OP.add)
            nc.sync.dma_start(out=y[:, :], in_=yt[:])

    nc.compile()
    return nc


def kernel(**inputs):
    hp, in_maps = _prep(inputs)
    nc = _build(hp)
    from concourse import bass_utils
    res = bass_utils.run_bass_kernel_spmd(nc, in_maps,
                                          core_ids=list(range(NCORES)))
    return np.asarray(res.results[0]["y"], np.float32)

